# revision 17
# baseline (speedup 1.0000x reference)
"""DCGRU cell (diffusion-conv GRU) Trainium2 Bass kernel — fp8 diffusion.

Sharding: data-parallel over batch B=32 across 8 NeuronCores (4 batches
per core). Support matrices and weights are replicated; no collectives.

Per-core layout (Bc=4 local batches, N=4096 nodes, U=64 units, F=66
features, M=5 diffusion matrices):
  - Feature order is [hx(64), inputs(2)] (NOT the reference order) so
    that gate outputs r/u stay partition-aligned with hx for the
    lane-locked elementwise engines. Weight rows are permuted to match.
  - "normal" layout: (N rows, Bc*F cols), col = b*66 + f.
  - "transposed" layout: (F rows, Bc*N cols), col = b*4096 + n.

The 8 large diffusion matmuls (S @ x, 4096x4096x264 each) run in
fp8e4m3 with MatmulPerfMode.DoubleRow: two 128-deep k-subtiles are
packed per instruction ([128, 2, free] APs), giving 2x PE throughput
vs bf16. S is pre-scaled by 64 (keeps entries out of fp8 subnormals);
x1 is stored as 8*x1 in fp8. Scales are undone in the PSUM->SBUF
copies and in host-side weight folds.

The Chebyshev feature x2 = 2*S@x1 - x0 is never materialized: the
dense GEMM uses z = 2*S@x1 with the x0 weight folded on host to
W0' = W0 - W2 - W4. Dense GEMMs and elementwise stay bf16/fp32.

Measured rel-L2 error vs fp32 reference in simulation: ~2.4e-3.
"""

import numpy as np
import ml_dtypes

import concourse.bass as bass
import concourse.mybir as mybir
import concourse.tile as tile
from concourse import bacc
from concourse.bass_utils import run_bass_kernel_spmd
from concourse.masks import make_identity

N = 4096
B = 32
NCORES = 8
BC = B // NCORES          # 4 batches per core
U = 64
IN_DIM = 2
F = U + IN_DIM            # 66 (feature order: hx first, then inputs)
M = 5                     # matrices: [x0, x1_s0, z_s0, x1_s1, z_s1]
P = 128
NBLK = N // P             # 32 row tiles
MG = 8                    # m-groups of 512 output rows
JN = BC * F               # 264 cols in normal layout
JT = BC * N               # 16384 cols in transposed layout
NF = 8                    # free tiles of 512 over N per batch
FW = 512

NPAIR = 16                # k-pairs (2 x 128 contraction per matmul)
G = 4                     # k-pairs per S.T DMA (512 KB per transfer)
NG = NPAIR // G           # 4 DMA groups per (mg, pass)
SCOL = G * 4 * 2 * P      # 4096 cols per S.T dram row

SSCALE = 64.0             # S stored as 64*S in fp8 (avoids subnormals)
X1SCALE = 8.0             # x1 stored as 8*x1 in fp8

BF16 = mybir.dt.bfloat16
FP8 = mybir.dt.float8e4
F32 = mybir.dt.float32
AF = mybir.ActivationFunctionType
ALU = mybir.AluOpType
DR = mybir.MatmulPerfMode.DoubleRow

_cache = {}
INTERLEAVE = True


def _pair(ap):
    return ap.rearrange("p (two j) -> p two j", two=2)


def _diffuse(nc, dpool, stpool, st_dram, xin_pairs, make_out, after_group=None):
    """out_m = S @ xin for 32 m-blocks via fp8 DoubleRow matmuls.

    xin_pairs: 16 APs [P, 2, JN] fp8 (k-pairs). S.T streams as 512KB
    DMAs; each stationary is a [P, 2, 128] fp8 pair tile.
    """
    for mg in range(MG):
        psums = [dpool.tile([P, JN], F32, name="dp", tag="dp") for _ in range(4)]
        for kg in range(NG):
            st = stpool.tile([P, SCOL], FP8, name="st", tag="st")
            r0 = (mg * NG + kg) * P
            eng = nc.sync if (mg * NG + kg) % 2 == 0 else nc.gpsimd
            eng.dma_start(out=st[:, :], in_=st_dram[r0:r0 + P, :])
            for g in range(G):
                t = kg * G + g
                for mi in range(4):
                    c0 = (g * 4 + mi) * 2 * P
                    nc.tensor.matmul(
                        psums[mi][:, :],
                        _pair(st[:, c0:c0 + 2 * P]),
                        xin_pairs[t],
                        start=(t == 0),
                        stop=(t == NPAIR - 1),
                        perf_mode=DR,
                    )
        for mi in range(4):
            make_out(mg * 4 + mi, psums[mi])
        if after_group is not None:
            after_group(mg)


def _build_kernel(loop_iters=1, stages=4):
    nc = bacc.Bacc(
        "TRN2",
        target_bir_lowering=False,
        debug=False,
        num_devices=NCORES,
    )

    st0_d = nc.dram_tensor("st0", [MG * NG * P, SCOL], FP8, kind="ExternalInput").ap()
    st1_d = nc.dram_tensor("st1", [MG * NG * P, SCOL], FP8, kind="ExternalInput").ap()
    x0n_d = nc.dram_tensor("x0n", [N, JN], FP8, kind="ExternalInput").ap()
    x0t_d = nc.dram_tensor("x0t", [F, JT], BF16, kind="ExternalInput").ap()
    xit_d = nc.dram_tensor("xit", [IN_DIM, JT], BF16, kind="ExternalInput").ap()
    hxt_d = nc.dram_tensor("hxt", [U, JT], BF16, kind="ExternalInput").ap()
    w1_d = nc.dram_tensor("w1", [F, M * 2 * U], BF16, kind="ExternalInput").ap()
    w2_d = nc.dram_tensor("w2", [F, M * U], BF16, kind="ExternalInput").ap()
    # fp8 weight pairs for DoubleRow GEMM: cols [m1|m3] then [m2|m4]
    w18_d = nc.dram_tensor("w18", [F, 4 * 2 * U], FP8, kind="ExternalInput").ap()
    w28_d = nc.dram_tensor("w28", [F, 4 * U], FP8, kind="ExternalInput").ap()
    b1_d = nc.dram_tensor("b1", [2 * U, 1], F32, kind="ExternalInput").ap()
    b2_d = nc.dram_tensor("b2", [U, 1], F32, kind="ExternalInput").ap()
    out_d = nc.dram_tensor("out", [U, JT], F32, kind="ExternalOutput").ap()

    import contextlib
    with tile.TileContext(nc) as tc:
        with (
            tc.tile_pool(name="x1z", bufs=1) as mats_pool,
            tc.tile_pool(name="stp", bufs=5) as stpool,
            tc.tile_pool(name="x0w", bufs=4) as x0w_pool,
            tc.tile_pool(name="stage", bufs=4) as stage_pool,
            tc.tile_pool(name="big", bufs=1) as big_pool,
            tc.tile_pool(name="consts", bufs=1) as cpool,
            tc.tile_pool(name="tmp", bufs=2) as tmp_pool,
            (tc.For_i(0, loop_iters, 1) if loop_iters > 1
             else contextlib.nullcontext()),
        ):
            # ---- constants ----
            ident = cpool.tile([P, P], BF16, name="ident", tag="ident")
            make_identity(nc, ident[:, :])
            w1 = cpool.tile([F, M * 2 * U], BF16, name="w1", tag="w1")
            nc.scalar.dma_start(out=w1[:, :], in_=w1_d[:, :])
            w2 = cpool.tile([F, M * U], BF16, name="w2", tag="w2")
            nc.scalar.dma_start(out=w2[:, :], in_=w2_d[:, :])
            w18 = cpool.tile([F, 4 * 2 * U], FP8, name="w18", tag="w18")
            nc.scalar.dma_start(out=w18[:, :], in_=w18_d[:, :])
            w28 = cpool.tile([F, 4 * U], FP8, name="w28", tag="w28")
            nc.scalar.dma_start(out=w28[:, :], in_=w28_d[:, :])
            b1 = cpool.tile([2 * U, 1], F32, name="b1", tag="b1")
            nc.scalar.dma_start(out=b1[:, :], in_=b1_d[:, :])
            b2 = cpool.tile([U, 1], F32, name="b2", tag="b2")
            nc.scalar.dma_start(out=b2[:, :], in_=b2_d[:, :])

            # tileB: rows 0:64 = hx.T (bf16), rows 64:128 = u.T (written in GEMM1)
            tb = big_pool.tile([P, JT], BF16, name="tileB", tag="tileB")
            # x0't: rows 0:64 = state2.T = r*hx (GEMM1 output), rows 64:66 = inputs.T
            x0pt = big_pool.tile([F, JT], BF16, name="x0pt", tag="x0pt")

            def load_x0_pairs():
                # 4 wide fp8 tiles of 8 n-blocks each -> 16 pair APs
                pairs = []
                for g in range(4):
                    wt = x0w_pool.tile([P, 8 * JN], FP8, name="x0w", tag="x0w")
                    nc.scalar.dma_start(
                        out=wt[:, :].rearrange("p (nb j) -> p nb j", j=JN),
                        in_=x0n_d[g * 8 * P:(g + 1) * 8 * P, :].rearrange(
                            "(nb p) j -> p nb j", p=P
                        ),
                    )
                    for u in range(4):
                        pairs.append(_pair(wt[:, 2 * u * JN:(2 * u + 2) * JN]))
                return pairs

            def gconv(x0_pairs, x0t_rhs, w, w8, bias, outdim, post,
                      do_gemm=True):
                """One graph convolution: fp8 diffusion + dense GEMM.

                x0_pairs: 16 APs [P, 2, JN] fp8 (m=0 diffusion rhs)
                x0t_rhs(b, nf) -> AP (66, 512) bf16: transposed m=0 rhs
                w: (66, M*outdim) bf16 (m=0 slice used), bias: (outdim, 1)
                w8: (66, 4*outdim) fp8 weight pairs [m1|m3] then [m2|m4]
                post(b, nf, psum, c0): consume GEMM output (outdim, 512)

                The m1..m4 GEMM contributions run as two fp8 DoubleRow
                matmuls: supports s0/s1 are packed as the two k-subtiles
                ([66, 2, od] weights x [66, 2, 512] features).
                """
                # mats[1..4] accessors: (nb, b) -> AP [P, F] bf16
                mat_get = [None] * 5

                def gemm_iter(gpool, tpool, b, nf):
                    c0 = b * N + nf * FW
                    pg = gpool.tile([outdim, FW], F32, name="pg", tag="pg")
                    stgs = []
                    for mi in range(1, 5):
                        getter = mat_get[mi]
                        pt = tpool.tile([F, FW], BF16, name="pt", tag="pt")
                        for i in range(4):
                            nb = nf * 4 + i
                            nc.tensor.transpose(
                                pt[0:F, i * P:(i + 1) * P],
                                getter(nb, b),
                                ident[:, :],
                            )
                        stg = stage_pool.tile([F, FW], BF16, name="stg",
                                              tag="stg")
                        nc.vector.tensor_copy(stg[:, :], pt[0:F, :])
                        stgs.append(stg)
                    nc.tensor.matmul(
                        pg[:, :], w[:, 0:outdim], x0t_rhs(b, nf),
                        start=True, stop=False,
                    )
                    for mi in range(4):
                        nc.tensor.matmul(
                            pg[:, :],
                            w[:, (mi + 1) * outdim:(mi + 2) * outdim],
                            stgs[mi][:, :],
                            start=False, stop=(mi == 3),
                        )
                    post(b, nf, pg, c0)

                for si, st_dram in enumerate((st0_d, st1_d)):
                    # ---- pass 1: x1 = S@x0 ----
                    # stored twice: 8*x1 fp8 pairs (pass-2 diffusion rhs,
                    # DVE) and x1 bf16 (dense-GEMM feature, ACT)
                    x1p = [mats_pool.tile([P, 2 * JN], FP8, name="x1p",
                                          tag="x1p", bufs=16)
                           for _ in range(NPAIR)]
                    x1b = [None] * NBLK

                    def mk_x1(mblk, ps, x1p=x1p, x1b=x1b):
                        t, h = mblk // 2, mblk % 2
                        nc.vector.tensor_scalar_mul(
                            x1p[t][:, h * JN:(h + 1) * JN], ps[:, :],
                            X1SCALE / SSCALE,
                        )
                        tbx = mats_pool.tile([P, JN], BF16, name="x1b",
                                             tag="x1b", bufs=64)
                        nc.scalar.activation(
                            tbx[:, :], ps[:, :], AF.Copy, scale=1.0 / SSCALE,
                        )
                        x1b[mblk] = tbx

                    with tc.tile_pool(name="dpsum", bufs=8, space="PSUM") as dpool:
                        _diffuse(nc, dpool, stpool, st_dram, x0_pairs, mk_x1)

                    x1_pairs = [_pair(t[:, :]) for t in x1p]
                    mat_get[1 + 2 * si] = (
                        lambda nb, b, x1b=x1b: x1b[nb][:, b * F:(b + 1) * F])

                    # ---- pass 2: z = 2*S@x1 bf16 ----
                    zt = [None] * NBLK

                    def mk_z(mblk, ps, zt=zt):
                        t = mats_pool.tile([P, JN], BF16, name="z",
                                           tag="z", bufs=64)
                        nc.vector.tensor_scalar_mul(
                            t[:, :], ps[:, :], 2.0 / (SSCALE * X1SCALE),
                        )
                        zt[mblk] = t

                    mat_get[2 + 2 * si] = (
                        lambda nb, b, zt=zt: zt[nb][:, b * F:(b + 1) * F])

                    last = (si == 1) and do_gemm and INTERLEAVE
                    if not last:
                        with tc.tile_pool(name="dpsum", bufs=8, space="PSUM") as dpool:
                            _diffuse(nc, dpool, stpool, st_dram, x1_pairs, mk_z)
                    else:
                        with (
                            tc.tile_pool(name="dpsum", bufs=4, space="PSUM") as dpool,
                            tc.tile_pool(name="gpsum", bufs=2, space="PSUM") as gpool,
                            tc.tile_pool(name="tpsum", bufs=2, space="PSUM") as tpool,
                        ):
                            def after_group(mg):
                                if mg >= 1:
                                    for b in range(BC):
                                        gemm_iter(gpool, tpool, b, mg - 1)

                            _diffuse(nc, dpool, stpool, st_dram, x1_pairs,
                                     mk_z, after_group=after_group)
                            for b in range(BC):
                                gemm_iter(gpool, tpool, b, NF - 1)

                if not do_gemm or INTERLEAVE:
                    return
                with (
                    tc.tile_pool(name="gpsum", bufs=2, space="PSUM") as gpool,
                    tc.tile_pool(name="tpsum", bufs=2, space="PSUM") as tpool,
                ):
                    for b in range(BC):
                        for nf in range(NF):
                            gemm_iter(gpool, tpool, b, nf)

            # ================= gconv 1 =================
            x0_pairs = load_x0_pairs()
            nc.scalar.dma_start(out=tb[0:U, :], in_=hxt_d[:, :])
            nc.scalar.dma_start(out=x0pt[U:F, :], in_=xit_d[:, :])

            def x0t_rhs1(b, nf):
                stg = stage_pool.tile([F, FW], BF16, name="x0trhs", tag="x0trhs", bufs=2)
                c0 = b * N + nf * FW
                nc.sync.dma_start(out=stg[:, :], in_=x0t_d[:, c0:c0 + FW])
                return stg[:, :]

            def post1(b, nf, pg, c0):
                # r rows 0:64, u rows 64:128 (partition-aligned with hx/ut)
                tr = tmp_pool.tile([U, FW], F32, name="tr", tag="tr")
                nc.scalar.activation(
                    tr[:, :], pg[0:U, :], AF.Sigmoid, bias=b1[0:U, :],
                )
                nc.vector.tensor_tensor(
                    x0pt[0:U, c0:c0 + FW], tr[:, :], tb[0:U, c0:c0 + FW],
                    ALU.mult,
                )
                nc.scalar.activation(
                    tb[U:P, c0:c0 + FW], pg[U:2 * U, :], AF.Sigmoid,
                    bias=b1[U:2 * U, :],
                )

            gconv(x0_pairs, x0t_rhs1, w1, w18, b1, 2 * U, post1,
                  do_gemm=(stages >= 2))

            # ====== rebuild normal-layout fp8 x0' pairs from x0't ======
            if stages >= 3:
                x0pp = [mats_pool.tile([P, 2 * JN], FP8, name="x0pp",
                                       tag="x0pp", bufs=16)
                        for _ in range(NPAIR)]
                with tc.tile_pool(name="tpsum2", bufs=3, space="PSUM") as t2pool:
                    for nb in range(NBLK):
                        t, h = nb // 2, nb % 2
                        for b in range(BC):
                            pt = t2pool.tile([P, F], BF16, name="pt2", tag="pt2")
                            c0 = b * N + nb * P
                            nc.tensor.transpose(
                                pt[:, 0:F],
                                x0pt[0:F, c0:c0 + P],
                                ident[0:F, 0:F],
                            )
                            nc.vector.tensor_copy(
                                x0pp[t][:, h * JN + b * F:h * JN + (b + 1) * F],
                                pt[:, 0:F],
                            )
                x0p_pairs = [_pair(t[:, :]) for t in x0pp]

            # ================= gconv 2 =================
            def x0t_rhs2(b, nf):
                c0 = b * N + nf * FW
                return x0pt[0:F, c0:c0 + FW]

            def post2(b, nf, pg, c0):
                tc_ = tmp_pool.tile([U, FW], F32, name="tc", tag="tc")
                nc.scalar.activation(
                    tc_[:, :], pg[0:U, :], AF.Tanh, bias=b2[0:U, :],
                )
                tu = tmp_pool.tile([U, FW], BF16, name="tu", tag="tu")
                nc.sync.dma_start(out=tu[:, :], in_=tb[U:P, c0:c0 + FW])
                td = tmp_pool.tile([U, FW], F32, name="td", tag="td")
                nc.vector.tensor_sub(td[:, :], tb[0:U, c0:c0 + FW], tc_[:, :])
                to = tmp_pool.tile([U, FW], F32, name="to", tag="to")
                nc.vector.tensor_tensor(to[:, :], tu[:, :], td[:, :], ALU.mult)
                nc.vector.tensor_add(to[:, :], to[:, :], tc_[:, :])
                nc.sync.dma_start(out=out_d[:, c0:c0 + FW], in_=to[:, :])

            if stages >= 4:
                gconv(x0p_pairs, x0t_rhs2, w2, w28, b2, U, post2)

    nc.compile()
    return nc


def _get_nc(loop_iters=1, stages=4):
    key = f"nc{loop_iters}s{stages}"
    if key not in _cache:
        _cache[key] = _build_kernel(loop_iters, stages)
    return _cache[key]


def _bf16(a):
    return np.asarray(a, dtype=ml_dtypes.bfloat16)


def _fp8(a):
    return np.asarray(np.clip(a, -240.0, 240.0), dtype=ml_dtypes.float8_e4m3)


def _swizzle_support_fp8(s):
    """S (N,N) f32 -> 64*S.T pair-packed fp8 as (MG*NG*P, SCOL).

    dram[(mg*NG+kg)*P + p, (g*4+mi)*256 + i*128 + m]
        = 64 * S.T[(2*(kg*G+g)+i)*P + p, mg*512 + mi*128 + m]
    Each DMA row is one partition's 4KB contiguous run (G k-pairs,
    4 m-tiles, both halves of each pair)."""
    st = np.ascontiguousarray(s.T) * np.float32(SSCALE)
    t = st.reshape(NG, G, 2, P, MG, 4, P).transpose(4, 0, 3, 1, 5, 2, 6)
    return _fp8(np.ascontiguousarray(t.reshape(MG * NG * P, SCOL)))


def kernel(inputs, hx, support0, support1, weight, biases, weight_2, biases_2):
    inputs = np.asarray(inputs, np.float32)
    hx = np.asarray(hx, np.float32)
    support0 = np.asarray(support0, np.float32)
    support1 = np.asarray(support1, np.float32)
    weight = np.asarray(weight, np.float32)
    biases = np.asarray(biases, np.float32)
    weight_2 = np.asarray(weight_2, np.float32)
    biases_2 = np.asarray(biases_2, np.float32)

    st0 = _swizzle_support_fp8(support0)
    st1 = _swizzle_support_fp8(support1)

    # weight rows: reference feature index phi = f_ref*M + m with
    # f_ref 0,1 = inputs, 2..65 = hx. Permute rows to the kernel's
    # [hx, inputs] feature order; fold the algebraic x2-elimination
    # (x2 = 2*S@x1 - x0 is never materialized: the GEMM uses z = 2*S@x1
    # with W2/W4 and the x0 weight becomes W0-W2-W4); cols = m*od + o.
    def prep_w(w, od):
        wr = w.reshape(F, M, od)
        wr = np.concatenate([wr[IN_DIM:], wr[:IN_DIM]], axis=0)  # (66, 5, od)
        wf = np.stack([
            wr[:, 0] - wr[:, 2] - wr[:, 4],
            wr[:, 1],
            wr[:, 2],
            wr[:, 3],
            wr[:, 4],
        ], axis=1)
        return _bf16(np.ascontiguousarray(wf.reshape(F, M * od)))

    w1 = prep_w(weight, 2 * U)
    w2 = prep_w(weight_2, U)

    # fp8 weight pairs for the DoubleRow GEMM: [m1|m3] then [m2|m4],
    # each pair laid out i*od + o (i = 0:s0, 1:s1)
    def prep_w8(w, od):
        wr = w.reshape(F, M, od)
        wr = np.concatenate([wr[IN_DIM:], wr[:IN_DIM]], axis=0)
        wp = np.concatenate([wr[:, 1], wr[:, 3], wr[:, 2], wr[:, 4]], axis=1)
        return _fp8(np.ascontiguousarray(wp))

    w18 = prep_w8(weight, 2 * U)
    w28 = prep_w8(weight_2, U)
    b1 = np.ascontiguousarray(biases.reshape(2 * U, 1))
    b2 = np.ascontiguousarray(biases_2.reshape(U, 1))

    in_maps = []
    for c in range(NCORES):
        xb = inputs[c * BC:(c + 1) * BC].reshape(BC, N, IN_DIM)
        hb = hx[c * BC:(c + 1) * BC].reshape(BC, N, U)
        feat = np.concatenate([hb, xb], axis=2)  # (BC, N, 66) hx-first
        x0n = _fp8(np.ascontiguousarray(
            feat.transpose(1, 0, 2).reshape(N, JN)))
        x0t = _bf16(np.ascontiguousarray(
            feat.transpose(2, 0, 1).reshape(F, JT)))
        xit = _bf16(np.ascontiguousarray(
            xb.transpose(2, 0, 1).reshape(IN_DIM, JT)))
        hxt = _bf16(np.ascontiguousarray(
            hb.transpose(2, 0, 1).reshape(U, JT)))
        in_maps.append({
            "st0": st0, "st1": st1, "x0n": x0n, "x0t": x0t,
            "xit": xit, "hxt": hxt, "w1": w1, "w2": w2,
            "w18": w18, "w28": w28, "b1": b1, "b2": b2,
        })

    nc = _get_nc()
    _cache["last_in_maps"] = in_maps
    res = run_bass_kernel_spmd(
        nc, in_maps, core_ids=list(range(NCORES)),
        trace=_cache.get("trace", False),
    )
    _cache["last_results"] = res

    out = np.empty((B, N * U), np.float32)
    for c in range(NCORES):
        oc = res.results[c]["out"]  # (U, BC*N) f32
        for b in range(BC):
            out[c * BC + b] = np.ascontiguousarray(
                oc[:, b * N:(b + 1) * N].T).reshape(N * U)
    return out
